# revision 11
# baseline (speedup 1.0000x reference)
"""Trainium2 Bass kernel for nn_Decoder_45260365365566 (gnn_message_passing).

Contract: kernel(**inputs) takes the FULL unsharded inputs from
reference.setup_inputs() and returns the FULL [B, N+O, D] output.

Strategy (8 NeuronCores): 2-way data parallel over batch B; within each batch
group, 4-way row-sharding of the [N, N] adjacencies (N=4096 -> 1024 rows per
core).  The sharding step hands each core its adjacency slices COLUMN-MAJOR
(transposed on the host while slicing): the TensorEngine contracts over the
partition axis, so feeding M^T tiles directly avoids any on-device transpose
of the ~335 MB adjacency stream.  Each core:
  - builds the small projected feature matrices Xk = all_w@Wwk+bwk,
    Xs = w@Wws+bws (replicated work, tiny), augmented with a ones column so
    the binary-adjacency row-degree falls out of the same matmul,
  - streams its (4096 x 1024) slice of (word_word * word_exist_matrix)^T and
    its (2048 x 1024) slice of depend^T, multiplying masks on VectorE (bf16
    out: masks are 0/1, exact) and accumulating  Y^T[h, i] += Xk^T M^T  on
    the TensorEngine in [65, 512] PSUM strips at full bf16 rate,
  - transposes the two [65, 1024] results back (8 tiny PE transposes),
    finishes word_g / word_w_k / word_w_s row-normalisation, the
    forget/update gating, and word_updated rows (1/8 of the full output),
  - computes its partial of word_op's token contraction P = A^T @ [Z, 1].
The host sums the four [128, 65] P partials per batch (a 33 KB reduction --
an on-device AllReduce measured ~65 us on this fabric, far more than it is
worth) and applies the tiny [128, 64] op-embedding epilogue in numpy.

All _norm_mat inputs are binary masks, so degree == row-sum, folded into the
matmul via the ones column.  l2n(x/(deg+eps)) == x/(||x||+eps') exactly up to
fp32 rounding, so normalisation happens once, after the matmul.
"""

import concurrent.futures

import numpy as np

B, L, D, H, O = 2, 2048, 64, 64, 128
N = 2 * L                # 4096 tokens
S = N // 4               # 1024 rows per core
NB = S // 128            # 8 row-blocks per core
NJ = N // 128            # 32 column-blocks of the big adjacency
NJS = L // 128           # 16 column-blocks of depend
FT_CLIP = 0.2
N_CORES = 8

_CACHED = {}


# --------------------------------------------------------------------------
# walrus workaround: this toolchain's walrus accepts at most ONE sync-wait
# command per instruction, but Tile emits several.  Legalize the serialized
# BIR by hoisting extra waits onto NoOps inserted before the owner (engines
# run a basic block in program order, so consecutive waits AND together).
# --------------------------------------------------------------------------
def _legalize_bir(d):
    def fix_list(insts):
        out = []
        for inst in insts:
            si = inst.get("sync_info")
            if si:
                waits = si.get("on_wait") or []
                if len(waits) > 1:
                    for k, w in enumerate(waits[:-1]):
                        out.append({
                            "debug": inst.get("debug", 0),
                            "engine": inst["engine"],
                            "ins": [], "outs": [],
                            "name": f"{inst['name']}-xw{k}",
                            "opcode": "NoOp",
                            "sync_info": {"on_update": [], "on_wait": [w]},
                        })
                    si["on_wait"] = waits[-1:]
            out.append(inst)
        return out

    def walk(o):
        if isinstance(o, dict):
            if isinstance(o.get("instructions"), list):
                o["instructions"] = fix_list(o["instructions"])
            for v in o.values():
                walk(v)
        elif isinstance(o, list):
            for v in o:
                walk(v)

    walk(d)
    return d


def _patch_to_json_bytes(nc):
    import orjson
    orig = nc.to_json_bytes

    def patched():
        return orjson.dumps(_legalize_bir(orjson.loads(orig())))

    nc.to_json_bytes = patched


# --------------------------------------------------------------------------
# device program (identical on all 8 cores)
# --------------------------------------------------------------------------
def _build_nc():
    import concourse.bass as bass
    import concourse.tile as tile
    from concourse import mybir
    from concourse.masks import make_identity

    FT = mybir.dt.float32
    BF = mybir.dt.bfloat16
    ACT = mybir.ActivationFunctionType

    nc = bass.Bass(num_devices=N_CORES)
    din = lambda name, shape: nc.dram_tensor(name, shape, FT, kind="ExternalInput")
    dout = lambda name, shape: nc.dram_tensor(name, shape, FT, kind="ExternalOutput")

    adjT = din("adjT", [N, S])        # wem rows for this shard, transposed
    wwT = din("wwT", [N, S])          # word_word rows, transposed
    depT = din("depT", [L, S])        # depend_{0|1} rows, transposed
    w0 = din("w0", [L, D])
    w1 = din("w1", [L, D])
    wfs = din("wfs", [L, D])          # w0 or w1 (matches dep's column space)
    awl = din("awl", [S, D])          # all_w rows of this shard
    wops = din("wops", [S, O])        # word_operator rows
    wesc = din("wesc", [128, NB])     # wes shard, column-major blocks
    gws = din("gws", [1, S])          # goal_word shard row
    nh = din("nh", [1, D])
    Wg = din("Wg", [D, H]); bg = din("bg", [1, H])
    Wwk = din("Wwk", [D, H]); bwk = din("bwk", [1, H])
    Wws = din("Wws", [D, H]); bws = din("bws", [1, H])
    Wo = din("Wo", [D, D]); bo = din("bo", [1, D])
    Wupd = din("Wupd", [3 * H, D]); bupd = din("bupd", [1, D])
    Wf = din("Wf", [D + 3 * H, D]); bf = din("bf", [1, D])

    out_w = dout("out_w", [S, D])
    out_p = dout("out_p", [128, 65])

    with tile.TileContext(nc) as tc:
        with (
            tc.tile_pool(name="cons", bufs=1) as cons,
            tc.tile_pool(name="stream", bufs=4) as stream,
            tc.tile_pool(name="ep", bufs=3) as ep,
            tc.tile_pool(name="ktok", bufs=1) as ktokp,
            tc.tile_pool(name="ps_acc", bufs=2, space="PSUM") as ps_acc,
            tc.tile_pool(name="ps_tp", bufs=3, space="PSUM") as ps_tp,
            tc.tile_pool(name="ps_ep", bufs=2, space="PSUM") as ps_ep,
            tc.tile_pool(name="ps_P", bufs=1, space="PSUM") as ps_P,
        ):
            ident = cons.tile([128, 128], FT)
            make_identity(nc, ident[:])

            # ---- small weight staging -------------------------------------
            def aug_square(name, Wsrc, bsrc):
                # [[W, 0], [b, 1]]  (65 x 65): ones column computes row-degree
                t = cons.tile([65, 65], FT, tag=name)
                nc.vector.memset(t[:], 0.0)
                nc.sync.dma_start(t[0:64, 0:64], Wsrc[:])
                nc.sync.dma_start(t[64:65, 0:64], bsrc[:])
                nc.vector.memset(t[64:65, 64:65], 1.0)
                return t

            def aug_rhs(name, Wsrc, bsrc):
                # [W; b]  (65 x 64): bias folded via a ones row in lhsT
                t = cons.tile([65, 64], FT, tag=name)
                nc.sync.dma_start(t[0:64, :], Wsrc[:])
                nc.sync.dma_start(t[64:65, :], bsrc[:])
                return t

            def plain_rhs(name, ap):
                t = cons.tile([64, 64], FT, tag=name)
                nc.sync.dma_start(t[:], ap)
                return t

            Rwk = aug_square("Rwk", Wwk, bwk)
            Rws = aug_square("Rws", Wws, bws)
            Ro = aug_square("Ro", Wo, bo)
            rhs_g = aug_rhs("rhs_g", Wg, bg)
            rhs_f0 = aug_rhs("rhs_f0", Wf[0:64, :], bf)
            rhs_f1 = plain_rhs("rhs_f1", Wf[64:128, :])
            rhs_f2 = plain_rhs("rhs_f2", Wf[128:192, :])
            rhs_f3 = plain_rhs("rhs_f3", Wf[192:256, :])
            rhs_u0 = plain_rhs("rhs_u0", Wupd[0:64, :])
            rhs_u1 = aug_rhs("rhs_u1", Wupd[64:128, :], bupd)
            rhs_u2 = plain_rhs("rhs_u2", Wupd[128:192, :])

            wesc_sb = cons.tile([128, NB], FT)
            nc.sync.dma_start(wesc_sb[:], wesc[:])

            # ---- transposed token embeddings ------------------------------
            def load_tokens(name, src, ntile):
                t = cons.tile([128, ntile * 64], FT, tag=name)
                for i in range(ntile):
                    nc.sync.dma_start(t[:, i * 64:(i + 1) * 64],
                                      src[i * 128:(i + 1) * 128, :])
                return t

            w0_sb = load_tokens("w0_sb", w0, L // 128)
            w1_sb = load_tokens("w1_sb", w1, L // 128)
            wfs_sb = load_tokens("wfs_sb", wfs, L // 128)
            awl_sb = load_tokens("awl_sb", awl, NB)

            def strip_tile(name, src_sb, j):
                # [65, 128] feature-major strip (+ones row) of token block j
                t = cons.tile([65, 128], FT, tag=name, name=name)
                tp = ps_tp.tile([64, 128], FT, tag="tp", name=f"tp_{name}")
                nc.tensor.transpose(tp[:], src_sb[:, j * 64:(j + 1) * 64],
                                    ident[:])
                nc.vector.tensor_copy(t[0:64, :], tp[:])
                nc.vector.memset(t[64:65, :], 1.0)
                return t

            def x_tile(name, strip, R):
                # [128, 65] bf16 projected+augmented features of one block
                t = cons.tile([128, 65], BF, tag=name, name=name)
                px = ps_tp.tile([128, 65], FT, tag="tp", name=f"px_{name}")
                nc.tensor.matmul(px[:], strip[:], R[:])
                nc.scalar.copy(t[:], px[:])
                return t

            wfsT = [strip_tile(f"wfsT{i}", wfs_sb, i) for i in range(NJS)]
            xs_tiles = [x_tile(f"xs{i}", wfsT[i], Rws) for i in range(NJS)]
            awT = ([strip_tile(f"awT{i}", w0_sb, i) for i in range(L // 128)]
                   + [strip_tile(f"awTb{i}", w1_sb, i) for i in range(L // 128)])
            xk_tiles = [x_tile(f"xk{i}", awT[i], Rwk) for i in range(NJ)]
            awlT = [strip_tile(f"awlT{i}", awl_sb, i) for i in range(NB)]

            # ---- word_g^T [64, 1024] --------------------------------------
            nh_row = cons.tile([1, 64], FT)
            nc.sync.dma_start(nh_row[:], nh[:])
            tpn = ps_tp.tile([64, 1], FT, tag="tp")
            nc.tensor.transpose(tpn[:], nh_row[:], ident[0:1, 0:1])
            nhT = cons.tile([65, 1], FT)
            nc.vector.tensor_copy(nhT[0:64, :], tpn[:])
            nc.vector.memset(nhT[64:65, :], 1.0)
            pgt = ps_tp.tile([1, 64], FT, tag="tp")
            nc.tensor.matmul(pgt[:], nhT[:], rhs_g[:])
            gt_row = cons.tile([1, 64], FT)
            nc.vector.tensor_copy(gt_row[:], pgt[:])
            gt_sq = cons.tile([1, 64], FT)
            gt_ss = cons.tile([1, 1], FT)
            nc.scalar.activation(gt_sq[:], pgt[:], ACT.Square, accum_out=gt_ss[:])
            gt_n = cons.tile([1, 1], FT)
            nc.scalar.sqrt(gt_n[:], gt_ss[:])
            gws_sb = cons.tile([1, S], FT)
            nc.sync.dma_start(gws_sb[:], gws[:])
            den = cons.tile([1, S], FT)
            nc.vector.tensor_scalar(den[:], gws_sb[:], gt_n[:], 1e-30,
                                    op0=mybir.AluOpType.mult,
                                    op1=mybir.AluOpType.add)
            rec = cons.tile([1, S], FT)
            nc.vector.reciprocal(rec[:], den[:])
            alpha = cons.tile([1, S], FT)
            nc.vector.tensor_mul(alpha[:], rec[:], gws_sb[:])
            gT = cons.tile([64, S], FT)
            for h in range(S // 512):
                pg = ps_tp.tile([64, 512], FT, tag="tp")
                nc.tensor.matmul(pg[:], gt_row[:],
                                 alpha[:, h * 512:(h + 1) * 512])
                nc.vector.tensor_copy(gT[:, h * 512:(h + 1) * 512], pg[:])

            # ---- streamed accumulation  Y^T = X^T M^T  --------------------
            # depend phase first (smaller), then the big adjacency phase
            # reusing the same two PSUM accumulator slots.
            ps_T = [ps_acc.tile([65, 512], FT, tag="acc", name=f"psT{u}") for u in range(2)]
            for j in range(NJS):
                dt_ = stream.tile([128, S], FT, tag="dT")
                nc.sync.dma_start(dt_[:], depT[j * 128:(j + 1) * 128, :])
                dbf = stream.tile([128, S], BF, tag="dTb")
                nc.scalar.copy(dbf[:], dt_[:])
                for u in range(2):
                    nc.tensor.matmul(ps_T[u][:],
                                     xs_tiles[j][:],
                                     dbf[:, u * 512:(u + 1) * 512],
                                     start=(j == 0), stop=(j == NJS - 1))
            ysT = [ep.tile([65, 512], FT, tag=f"ysT{u}", name=f"ysT{u}") for u in range(2)]
            for u in range(2):
                nc.vector.tensor_copy(ysT[u][:], ps_T[u][:])

            pk_T = [ps_acc.tile([65, 512], FT, tag="acc", name=f"pkT{u}") for u in range(2)]
            for j in range(NJ):
                at = stream.tile([128, S], FT, tag="adjT")
                nc.sync.dma_start(at[:], adjT[j * 128:(j + 1) * 128, :])
                bt = stream.tile([128, S], FT, tag="wwT")
                nc.sync.dma_start(bt[:], wwT[j * 128:(j + 1) * 128, :])
                mt = stream.tile([128, S], BF, tag="mT")
                nc.vector.tensor_mul(mt[:], at[:], bt[:])
                for u in range(2):
                    nc.tensor.matmul(pk_T[u][:],
                                     xk_tiles[j][:],
                                     mt[:, u * 512:(u + 1) * 512],
                                     start=(j == 0), stop=(j == NJ - 1))
            ykT = [ep.tile([65, 512], FT, tag=f"ykT{u}", name=f"ykT{u}") for u in range(2)]
            for u in range(2):
                nc.vector.tensor_copy(ykT[u][:], pk_T[u][:])

            # ---- helpers --------------------------------------------------
            def l2n_rows(py, tagp):
                sq = ep.tile([128, 64], FT, tag=f"sq{tagp}")
                ss = ep.tile([128, 1], FT, tag=f"ss{tagp}")
                nc.scalar.activation(sq[:], py[:, 0:64], ACT.Square,
                                     accum_out=ss[:])
                nrm = ep.tile([128, 1], FT, tag=f"nrm{tagp}")
                nc.scalar.sqrt(nrm[:], ss[:])
                nc.vector.tensor_scalar_add(nrm[:], nrm[:], 1e-30)
                rin = ep.tile([128, 1], FT, tag=f"rin{tagp}")
                nc.vector.reciprocal(rin[:], nrm[:])
                res = ep.tile([128, 64], FT, tag=f"res{tagp}")
                nc.vector.tensor_scalar_mul(res[:], py[:, 0:64], rin[:])
                return res

            def strip_T(tok_tile, with_ones, tagp):
                tp = ps_tp.tile([64, 128], FT, tag="tp")
                nc.tensor.transpose(tp[:], tok_tile[:], ident[:])
                st = ep.tile([65, 128], FT, tag=f"st{tagp}")
                nc.vector.tensor_copy(st[0:64, :], tp[:])
                if with_ones:
                    nc.vector.memset(st[64:65, :], 1.0)
                return st

            # ---- per-row-block epilogue -----------------------------------
            pP = ps_P.tile([128, 65], FT)
            k_toks = [ktokp.tile([128, 64], FT, tag=f"kt{ib}", name=f"kt{ib}") for ib in range(NB)]
            for ib in range(NB):
                u, q = divmod(ib, 4)
                pyk = ps_ep.tile([128, 64], FT, tag="ep")
                nc.tensor.transpose(pyk[:], ykT[u][0:64, q * 128:(q + 1) * 128],
                                    ident[0:64, 0:64])
                nc.vector.tensor_copy(k_toks[ib][:], l2n_rows(pyk, "k")[:])

            for ib in range(NB):
                r0 = ib * 128
                u, q = divmod(ib, 4)
                k_tok = k_toks[ib]
                pys = ps_ep.tile([128, 64], FT, tag="ep")
                nc.tensor.transpose(pys[:], ysT[u][0:64, q * 128:(q + 1) * 128],
                                    ident[0:64, 0:64])
                s_tok = l2n_rows(pys, "s")
                kT = strip_T(k_tok, True, "k")
                sT = strip_T(s_tok, False, "s")

                pf = ps_ep.tile([128, 64], FT, tag="ep")
                nc.tensor.matmul(pf[:], awlT[ib][:], rhs_f0[:],
                                 start=True, stop=False)
                nc.tensor.matmul(pf[:], gT[:, r0:r0 + 128], rhs_f1[:],
                                 start=False, stop=False)
                nc.tensor.matmul(pf[:], kT[0:64, :], rhs_f2[:],
                                 start=False, stop=False)
                nc.tensor.matmul(pf[:], sT[0:64, :], rhs_f3[:],
                                 start=False, stop=True)
                pn = ps_ep.tile([128, 64], FT, tag="ep")
                nc.tensor.matmul(pn[:], gT[:, r0:r0 + 128], rhs_u0[:],
                                 start=True, stop=False)
                nc.tensor.matmul(pn[:], kT[:], rhs_u1[:],
                                 start=False, stop=False)
                nc.tensor.matmul(pn[:], sT[0:64, :], rhs_u2[:],
                                 start=False, stop=True)

                f_sb = ep.tile([128, 64], FT, tag="f")
                nc.scalar.activation(f_sb[:], pf[:], ACT.Sigmoid)
                r_sb = ep.tile([128, 64], FT, tag="r")
                nc.scalar.activation(r_sb[:], pn[:], ACT.Relu)
                fm = ep.tile([128, 64], FT, tag="fm")
                nc.vector.tensor_scalar_max(fm[:], f_sb[:], FT_CLIP)
                omf = ep.tile([128, 64], FT, tag="omf")
                nc.scalar.activation(omf[:], f_sb[:], ACT.Copy,
                                     bias=1.0, scale=-1.0)
                t1 = ep.tile([128, 64], FT, tag="t1")
                nc.vector.tensor_mul(t1[:], fm[:],
                                     awl_sb[:, ib * 64:(ib + 1) * 64])
                t2 = ep.tile([128, 64], FT, tag="t2")
                nc.vector.tensor_mul(t2[:], omf[:], r_sb[:])
                wu = ep.tile([128, 64], FT, tag="wu")
                nc.vector.tensor_add(wu[:], t1[:], t2[:])
                nc.sync.dma_start(out_w[r0:r0 + 128, :], wu[:])

                # op-stage partial: P += A^T @ [Z, 1]
                wuT = strip_T(wu, True, "wu")
                pz = ps_ep.tile([128, 65], FT, tag="ep")
                nc.tensor.matmul(pz[:], wuT[:], Ro[:])
                z_sb = ep.tile([128, 65], FT, tag="z")
                nc.vector.tensor_copy(z_sb[:], pz[:])
                wop_t = ep.tile([128, O], FT, tag="wop")
                nc.sync.dma_start(wop_t[:], wops[r0:r0 + 128, :])
                a_m = ep.tile([128, O], FT, tag="am")
                nc.vector.tensor_scalar_mul(a_m[:], wop_t[:],
                                            wesc_sb[:, ib:ib + 1])
                nc.tensor.matmul(pP[:], a_m[:], z_sb[:],
                                 start=(ib == 0), stop=(ib == NB - 1))

            p_sb = ep.tile([128, 65], FT, tag="psb")
            nc.vector.tensor_copy(p_sb[:], pP[:])
            nc.sync.dma_start(out_p[:], p_sb[:])

    _patch_to_json_bytes(nc)
    return nc


def _get_nc():
    if "nc" not in _CACHED:
        _CACHED["nc"] = _build_nc()
    return _CACHED["nc"]


# --------------------------------------------------------------------------
# host side: shard (transposing the adjacency slices), run, gather,
# tiny op epilogue
# --------------------------------------------------------------------------
def _sigmoid(x):
    return 1.0 / (1.0 + np.exp(-x))


def _core_map(inputs, c, shared):
    b, s = divmod(c, 4)
    r0, r1 = s * S, (s + 1) * S
    f32 = lambda a: np.ascontiguousarray(a, dtype=np.float32)
    dep = (inputs["depend_0"][b, r0:r1, :] if s < 2
           else inputs["depend_1"][b, r0 - L:r1 - L, :])
    m = {
        "adjT": f32(np.asarray(inputs["word_exist_matrix"][b, r0:r1, :]).T),
        "wwT": f32(np.asarray(inputs["word_word"][b, r0:r1, :]).T),
        "depT": f32(np.asarray(dep).T),
        "w0": f32(inputs["word_outputs_0"][b]),
        "w1": f32(inputs["word_outputs_1"][b]),
        "wfs": f32(inputs["word_outputs_0"][b] if s < 2
                   else inputs["word_outputs_1"][b]),
        "awl": f32(inputs["word_outputs_0"][b][r0:r1] if s < 2
                   else inputs["word_outputs_1"][b][r0 - L:r1 - L]),
        "wops": f32(inputs["word_operator"][b, r0:r1, :]),
        "wesc": f32(np.asarray(inputs["word_exist_sequence"][b, r0:r1])
                    .reshape(NB, 128).T),
        "gws": f32(np.asarray(inputs["goal_word"][b, r0:r1]).reshape(1, S)),
        "nh": f32(np.asarray(inputs["node_hidden"][b]).reshape(1, D)),
    }
    m.update(shared)
    return m


def _run(inputs, trace=False):
    from concourse.bass_utils import run_bass_kernel_spmd

    nc = _get_nc()
    f32 = lambda a: np.ascontiguousarray(a, dtype=np.float32)

    shared = {}
    for nm in ["Wg", "bg", "Wwk", "bwk", "Wws", "bws", "Wo", "bo",
               "Wupd", "bupd", "Wf", "bf"]:
        a = np.asarray(inputs[nm])
        shared[nm] = f32(a.reshape(1, -1) if a.ndim == 1 else a)

    with concurrent.futures.ThreadPoolExecutor(max_workers=8) as ex:
        in_maps = list(ex.map(lambda c: _core_map(inputs, c, shared),
                              range(N_CORES)))

    res = run_bass_kernel_spmd(nc, in_maps, core_ids=list(range(N_CORES)),
                               trace=trace)

    out = np.empty((B, N + O, D), dtype=np.float32)
    for c in range(N_CORES):
        b, s = divmod(c, 4)
        out[b, s * S:(s + 1) * S, :] = res.results[c]["out_w"]

    # host op epilogue (tiny): P summed over each batch's 4 shards
    opE = np.asarray(inputs["op_embedding"], dtype=np.float32)
    Wf2 = np.asarray(inputs["Wf2"], dtype=np.float32)
    bf2 = np.asarray(inputs["bf2"], dtype=np.float32)
    Wout = np.asarray(inputs["Wout"], dtype=np.float32)
    bout = np.asarray(inputs["bout"], dtype=np.float32)
    for b in range(B):
        P = np.zeros((128, 65), dtype=np.float32)
        for s in range(4):
            P += res.results[4 * b + s]["out_p"]
        word_op = P[:, 0:64] / (P[:, 64:65] + 1e-30)
        f2 = _sigmoid(np.concatenate([opE[b], word_op], axis=-1) @ Wf2 + bf2)
        r2 = np.maximum(word_op @ Wout + bout, 0.0)
        out[b, N:, :] = np.maximum(f2, FT_CLIP) * opE[b] + (1.0 - f2) * r2

    return out, res


def kernel(**inputs):
    out, _ = _run(inputs, trace=False)
    return out


# revision 16
# speedup vs baseline: 1.7597x; 1.7597x over previous
"""Trainium2 Bass kernel for nn_Decoder_45260365565566-family (gnn_message_passing).

Contract: kernel(**inputs) takes the FULL unsharded inputs from
reference.setup_inputs() and returns the FULL [B, N+O, D] output.

Strategy (8 NeuronCores): 2-way data parallel over batch B; within each batch
group, 4-way row-sharding of the [N, N] adjacencies (N=4096 -> 1024 rows per
core).  The sharding step hands each core its adjacency slices COLUMN-MAJOR
(transposed while slicing) and in bf16 (the masks are 0/1 -> exact recode,
halving the dominant DMA stream): the TensorEngine contracts over the
partition axis, so feeding M^T tiles directly avoids any on-device transpose
of the adjacency stream.  Each core:
  - streams its (4096 x 1024) slice of (word_word * word_exist_matrix)^T and
    its (2048 x 1024) slice of depend^T, multiplying masks on VectorE and
    accumulating  Y^T[h, i] += X^T M^T  on the TensorEngine in [65, 512]
    PSUM strips at bf16 rate (X carries a ones column so the binary
    row-degree falls out of the same matmul),
  - computes the word_g / word_w_k / word_w_s row normalisation and the
    forget/update gating with batched, weight-stationary bf16 matmuls in the
    feature-major domain, producing word_updated rows (1/8 of the output),
  - computes its partial of word_op's token contraction P = A^T @ [Z, 1].
The host sums the four [128, 65] P partials per batch (a 33 KB reduction --
an on-device AllReduce measured ~65 us on this fabric) and applies the tiny
[128, 64] op-embedding epilogue in numpy.

l2n(x/(deg+eps)) == x/(||x||+eps') up to fp32 rounding, so normalisation
happens once, after the matmul; norms and gating elementwise stay fp32.
"""

import concurrent.futures

import numpy as np

B, L, D, H, O = 2, 2048, 64, 64, 128
N = 2 * L                # 4096 tokens
S = N // 4               # 1024 rows per core
NB = S // 128            # 8 row-blocks per core
NJ = N // 128            # 32 column-blocks of the big adjacency
NJS = L // 128           # 16 column-blocks of depend
FT_CLIP = 0.2
N_CORES = 8

# packed weight image: [65, 65] column-blocks; cols 0:64 = [W; b], col 64 =
# [0...0, 1] for the "R" squares (ones column -> degree), else 0.
_CHUNKS = ["Rwk", "Rws", "Ro", "G", "F0", "F1", "F2", "F3", "U0", "U1", "U2"]

_CACHED = {}


# --------------------------------------------------------------------------
# walrus workaround: this toolchain's walrus accepts at most ONE sync-wait
# command per instruction, but Tile emits several.  Legalize the serialized
# BIR by hoisting extra waits onto NoOps inserted before the owner (engines
# run a basic block in program order, so consecutive waits AND together).
# --------------------------------------------------------------------------
def _legalize_bir(d):
    def fix_list(insts):
        out = []
        for inst in insts:
            si = inst.get("sync_info")
            if si:
                waits = si.get("on_wait") or []
                if len(waits) > 1:
                    for k, w in enumerate(waits[:-1]):
                        out.append({
                            "debug": inst.get("debug", 0),
                            "engine": inst["engine"],
                            "ins": [], "outs": [],
                            "name": f"{inst['name']}-xw{k}",
                            "opcode": "NoOp",
                            "sync_info": {"on_update": [], "on_wait": [w]},
                        })
                    si["on_wait"] = waits[-1:]
            out.append(inst)
        return out

    def walk(o):
        if isinstance(o, dict):
            if isinstance(o.get("instructions"), list):
                o["instructions"] = fix_list(o["instructions"])
            for v in o.values():
                walk(v)
        elif isinstance(o, list):
            for v in o:
                walk(v)

    walk(d)
    return d


def _patch_to_json_bytes(nc):
    import orjson
    orig = nc.to_json_bytes

    def patched():
        return orjson.dumps(_legalize_bir(orjson.loads(orig())))

    nc.to_json_bytes = patched


# --------------------------------------------------------------------------
# device program (identical on all 8 cores)
# --------------------------------------------------------------------------
def _build_nc():
    import concourse.bass as bass
    import concourse.tile as tile
    from concourse import mybir
    from concourse.masks import make_identity

    FT = mybir.dt.float32
    BF = mybir.dt.bfloat16
    ACT = mybir.ActivationFunctionType

    nc = bass.Bass(num_devices=N_CORES)

    adjT = nc.dram_tensor("adjT", [N, S], BF, kind="ExternalInput")
    wwT = nc.dram_tensor("wwT", [N, S], BF, kind="ExternalInput")
    depT = nc.dram_tensor("depT", [L, S], BF, kind="ExternalInput")
    wops = nc.dram_tensor("wops", [S, O], BF, kind="ExternalInput")
    wesc = nc.dram_tensor("wesc", [128, NB], FT, kind="ExternalInput")
    wpack = nc.dram_tensor("wpack", [65, 65 * len(_CHUNKS)], BF,
                           kind="ExternalInput")
    w0 = nc.dram_tensor("w0", [L, D], FT, kind="ExternalInput")
    w1 = nc.dram_tensor("w1", [L, D], FT, kind="ExternalInput")
    wfs = nc.dram_tensor("wfs", [L, D], FT, kind="ExternalInput")
    awl = nc.dram_tensor("awl", [S, D], FT, kind="ExternalInput")
    gws = nc.dram_tensor("gws", [1, S], FT, kind="ExternalInput")
    nh = nc.dram_tensor("nh", [1, D], FT, kind="ExternalInput")

    out_w = nc.dram_tensor("out_w", [S, D], FT, kind="ExternalOutput")
    out_p = nc.dram_tensor("out_p", [128, 65], FT, kind="ExternalOutput")

    with tile.TileContext(nc) as tc:
        with (
            tc.tile_pool(name="cons", bufs=1) as cons,
            tc.tile_pool(name="stream", bufs=6) as stream,
            tc.tile_pool(name="ep", bufs=2) as ep,
            tc.tile_pool(name="ps_acc", bufs=2, space="PSUM") as ps_acc,
            tc.tile_pool(name="ps_tp", bufs=3, space="PSUM") as ps_tp,
            tc.tile_pool(name="ps_ss", bufs=1, space="PSUM") as ps_ss,
            tc.tile_pool(name="ps_P", bufs=1, space="PSUM") as ps_P,
        ):
            # ===== A. stream loads + mask multiply (emitted first so the
            # DMA queues saturate from t=0; matmuls reference these later) ==
            d_tiles = []
            for j in range(NJS):
                dbf = stream.tile([128, S], BF, tag="dT", name=f"dT{j}")
                nc.sync.dma_start(dbf[:], depT[j * 128:(j + 1) * 128, :])
                d_tiles.append(dbf)
            m_tiles = []
            for j in range(NJ):
                at = stream.tile([128, S], BF, tag="adjT", name=f"at{j}")
                nc.sync.dma_start(at[:], adjT[j * 128:(j + 1) * 128, :])
                bt = stream.tile([128, S], BF, tag="wwT", name=f"bt{j}")
                nc.sync.dma_start(bt[:], wwT[j * 128:(j + 1) * 128, :])
                mt = stream.tile([128, S], BF, tag="mT", name=f"mt{j}")
                nc.vector.tensor_mul(mt[:], at[:], bt[:])
                m_tiles.append(mt)

            # ===== B. prep: weights, tokens, strips, X tiles ===============
            ident = cons.tile([128, 128], FT)
            make_identity(nc, ident[:])
            ident_bf = cons.tile([128, 128], BF)
            make_identity(nc, ident_bf[:])
            wp = cons.tile([65, 65 * len(_CHUNKS)], BF)
            nc.sync.dma_start(wp[:], wpack[:])
            CH = {nm: wp[:, i * 65:i * 65 + 65] for i, nm in enumerate(_CHUNKS)}
            W64 = {nm: CH[nm][:, 0:64] for nm in _CHUNKS}

            wesc_sb = cons.tile([128, NB], FT)
            nc.sync.dma_start(wesc_sb[:], wesc[:])

            def load_tokens(name, src, ntile):
                t = cons.tile([128, ntile * 64], FT, tag=name, name=name)
                nc.sync.dma_start(
                    t[:].rearrange("p (t d) -> p t d", d=64),
                    src[:].rearrange("(t p) d -> p t d", p=128))
                return t

            w0_sb = load_tokens("w0_sb", w0, L // 128)
            w1_sb = load_tokens("w1_sb", w1, L // 128)
            wfs_sb = load_tokens("wfs_sb", wfs, L // 128)
            awl_sb = load_tokens("awl_sb", awl, NB)

            def strips65(name, src_sb, ntile):
                # [65, ntile*128] bf16 feature-major strips (+ones row)
                big = cons.tile([65, ntile * 128], BF, tag=name, name=name)
                nc.vector.memset(big[64:65, :], 1.0)
                for i in range(ntile):
                    tp = ps_tp.tile([64, 128], FT, tag="tp",
                                    name=f"tp_{name}{i}")
                    nc.tensor.transpose(tp[:], src_sb[:, i * 64:(i + 1) * 64],
                                        ident[:])
                    if i % 2 == 0:
                        nc.vector.tensor_copy(
                            big[0:64, i * 128:(i + 1) * 128], tp[:])
                    else:
                        nc.scalar.copy(
                            big[0:64, i * 128:(i + 1) * 128], tp[:])
                return big

            def x_tile(name, strip, Rt):
                # [128, 65] bf16 projected+augmented features of one block
                t = cons.tile([128, 65], BF, tag=name, name=name)
                px = ps_tp.tile([128, 65], FT, tag="tp", name=f"px_{name}")
                nc.tensor.matmul(px[:], strip, Rt)
                nc.scalar.copy(t[:], px[:])
                return t

            wfsT = strips65("wfsT", wfs_sb, NJS)
            xs_tiles = [x_tile(f"xs{i}", wfsT[:, i * 128:(i + 1) * 128],
                               CH["Rws"]) for i in range(NJS)]
            awT0 = strips65("awT0", w0_sb, L // 128)
            awT1 = strips65("awT1", w1_sb, L // 128)
            xk_tiles = (
                [x_tile(f"xk{i}", awT0[:, i * 128:(i + 1) * 128], CH["Rwk"])
                 for i in range(L // 128)]
                + [x_tile(f"xkb{i}", awT1[:, i * 128:(i + 1) * 128], CH["Rwk"])
                   for i in range(L // 128)])
            awlT = strips65("awlT", awl_sb, NB)   # [65, 1024]

            ones_col = cons.tile([64, 1], FT)
            nc.vector.memset(ones_col[:], 1.0)

            # word_g^T [64, 1024] bf16 (g only enters the gate matmuls)
            nh_row = cons.tile([1, 64], FT)
            nc.sync.dma_start(nh_row[:], nh[:])
            tpn = ps_tp.tile([64, 1], FT, tag="tp")
            nc.tensor.transpose(tpn[:], nh_row[:], ident[0:1, 0:1])
            nhT = cons.tile([65, 1], BF)
            nc.vector.tensor_copy(nhT[0:64, :], tpn[:])
            nc.vector.memset(nhT[64:65, :], 1.0)
            pgt = ps_tp.tile([1, 64], FT, tag="tp")
            nc.tensor.matmul(pgt[:], nhT[:], W64["G"])
            gt_row = cons.tile([1, 64], BF)
            nc.vector.tensor_copy(gt_row[:], pgt[:])
            gt_sq = cons.tile([1, 64], FT)
            gt_ss = cons.tile([1, 1], FT)
            nc.scalar.activation(gt_sq[:], pgt[:], ACT.Square,
                                 accum_out=gt_ss[:])
            gt_n = cons.tile([1, 1], FT)
            nc.scalar.sqrt(gt_n[:], gt_ss[:])
            gws_sb = cons.tile([1, S], FT)
            nc.sync.dma_start(gws_sb[:], gws[:])
            den = cons.tile([1, S], FT)
            nc.vector.tensor_scalar(den[:], gws_sb[:], gt_n[:], 1e-30,
                                    op0=mybir.AluOpType.mult,
                                    op1=mybir.AluOpType.add)
            rec = cons.tile([1, S], FT)
            nc.vector.reciprocal(rec[:], den[:])
            alpha = cons.tile([1, S], BF)
            nc.vector.tensor_mul(alpha[:], rec[:], gws_sb[:])
            gT = cons.tile([64, S], BF)
            for hh in range(S // 512):
                pg = ps_tp.tile([64, 512], FT, tag="tp", name=f"pg{hh}")
                nc.tensor.matmul(pg[:], gt_row[:],
                                 alpha[:, hh * 512:(hh + 1) * 512])
                nc.vector.tensor_copy(gT[:, hh * 512:(hh + 1) * 512], pg[:])

            # ===== C/D. streamed accumulation  Y^T = X^T M^T ===============
            ps_T = [ps_acc.tile([65, 512], FT, tag="acc", name=f"psT{u}")
                    for u in range(2)]
            for j in range(NJS):
                for u in range(2):
                    nc.tensor.matmul(ps_T[u][:], xs_tiles[j][:],
                                     d_tiles[j][:, u * 512:(u + 1) * 512],
                                     start=(j == 0), stop=(j == NJS - 1))
            ysT = [ep.tile([65, 512], FT, tag=f"ysT{u}", name=f"ysT{u}")
                   for u in range(2)]
            for u in range(2):
                nc.vector.tensor_copy(ysT[u][:], ps_T[u][:])

            pk_T = [ps_acc.tile([65, 512], FT, tag="acc", name=f"pkT{u}")
                    for u in range(2)]
            for j in range(NJ):
                for u in range(2):
                    nc.tensor.matmul(pk_T[u][:], xk_tiles[j][:],
                                     m_tiles[j][:, u * 512:(u + 1) * 512],
                                     start=(j == 0), stop=(j == NJ - 1))
            ykT = [ep.tile([65, 512], FT, tag=f"ykT{u}", name=f"ykT{u}")
                   for u in range(2)]
            for u in range(2):
                nc.vector.tensor_copy(ykT[u][:], pk_T[u][:])

            # ===== E. l2 norms (pss cols: k = 0..7, s = 8..15) =============
            pss = ps_ss.tile([128, 16], FT)
            for u in range(2):
                sqk = ep.tile([64, 512], FT, tag="sqk", name=f"sqk{u}")
                nc.vector.tensor_mul(sqk[:], ykT[u][0:64, :], ykT[u][0:64, :])
                sqs = ep.tile([64, 512], FT, tag="sqs", name=f"sqs{u}")
                nc.vector.tensor_mul(sqs[:], ysT[u][0:64, :], ysT[u][0:64, :])
                for q in range(4):
                    ib = u * 4 + q
                    nc.tensor.matmul(pss[:, ib:ib + 1],
                                     sqk[:, q * 128:(q + 1) * 128],
                                     ones_col[:])
                    nc.tensor.matmul(pss[:, 8 + ib:9 + ib],
                                     sqs[:, q * 128:(q + 1) * 128],
                                     ones_col[:])
            nrm = ep.tile([128, 16], FT, tag="nrm")
            nc.scalar.sqrt(nrm[:], pss[:])
            nc.vector.tensor_scalar_add(nrm[:], nrm[:], 1e-30)
            rinv = ep.tile([128, 16], FT, tag="rinv")
            nc.vector.reciprocal(rinv[:], nrm[:])

            # token-major normalised k/s per block
            k_toks, s_toks = [], []
            for ib in range(NB):
                u, q = divmod(ib, 4)
                pyk = ps_tp.tile([128, 64], FT, tag="tp", name=f"pyk{ib}")
                nc.tensor.transpose(pyk[:],
                                    ykT[u][0:64, q * 128:(q + 1) * 128],
                                    ident[0:64, 0:64])
                kt = cons.tile([128, 64], FT, tag=f"ktok{ib}", name=f"ktok{ib}")
                nc.vector.tensor_scalar_mul(kt[:], pyk[:], rinv[:, ib:ib + 1])
                k_toks.append(kt)
                pys = ps_tp.tile([128, 64], FT, tag="tp", name=f"pys{ib}")
                nc.tensor.transpose(pys[:],
                                    ysT[u][0:64, q * 128:(q + 1) * 128],
                                    ident[0:64, 0:64])
                st = cons.tile([128, 64], FT, tag=f"stok{ib}", name=f"stok{ib}")
                nc.vector.tensor_scalar_mul(st[:], pys[:],
                                            rinv[:, 8 + ib:9 + ib])
                s_toks.append(st)

            # ===== F. feature-major strips of normalised k/s ===============
            wkT = cons.tile([65, S], BF, tag="wkT", name="wkT")
            nc.vector.memset(wkT[64:65, :], 1.0)
            wsT = cons.tile([65, S], BF, tag="wsT", name="wsT")
            nc.vector.memset(wsT[64:65, :], 1.0)
            for ib in range(NB):
                tpk = ps_tp.tile([64, 128], FT, tag="tp", name=f"tpk{ib}")
                nc.tensor.transpose(tpk[:], k_toks[ib][:], ident[:])
                nc.vector.tensor_copy(wkT[0:64, ib * 128:(ib + 1) * 128],
                                      tpk[:])
                tps = ps_tp.tile([64, 128], FT, tag="tp", name=f"tps{ib}")
                nc.tensor.transpose(tps[:], s_toks[ib][:], ident[:])
                nc.scalar.copy(wsT[0:64, ib * 128:(ib + 1) * 128], tps[:])

            # ===== G. gates, word_updated, op partials =====================
            wuT = cons.tile([65, S], BF, tag="wuT", name="wuT")
            nc.vector.memset(wuT[64:65, :], 1.0)
            for u in range(2):
                c0 = u * 512
                pf = ps_acc.tile([64, 512], FT, tag="acc", name=f"pfT{u}")
                nc.tensor.matmul(pf[:], W64["F0"], awlT[:, c0:c0 + 512],
                                 start=True, stop=False)
                nc.tensor.matmul(pf[:], W64["F1"][0:64, :],
                                 gT[:, c0:c0 + 512],
                                 start=False, stop=False)
                nc.tensor.matmul(pf[:], W64["F2"][0:64, :],
                                 wkT[0:64, c0:c0 + 512],
                                 start=False, stop=False)
                nc.tensor.matmul(pf[:], W64["F3"][0:64, :],
                                 wsT[0:64, c0:c0 + 512],
                                 start=False, stop=True)
                fT = ep.tile([64, 512], FT, tag="fT", name=f"fT{u}")
                nc.scalar.activation(fT[:], pf[:], ACT.Sigmoid)

                pn = ps_acc.tile([64, 512], FT, tag="acc", name=f"pnT{u}")
                nc.tensor.matmul(pn[:], W64["U0"][0:64, :],
                                 gT[:, c0:c0 + 512],
                                 start=True, stop=False)
                nc.tensor.matmul(pn[:], W64["U1"], wkT[:, c0:c0 + 512],
                                 start=False, stop=False)
                nc.tensor.matmul(pn[:], W64["U2"][0:64, :],
                                 wsT[0:64, c0:c0 + 512],
                                 start=False, stop=True)
                rT = ep.tile([64, 512], FT, tag="rT", name=f"rT{u}")
                nc.vector.tensor_scalar_max(rT[:], pn[:], 0.0)

                for q in range(4):
                    ib = u * 4 + q
                    r0 = ib * 128
                    ptf = ps_tp.tile([128, 64], FT, tag="tp", name=f"ptf{ib}")
                    nc.tensor.transpose(ptf[:], fT[:, q * 128:(q + 1) * 128],
                                        ident[0:64, 0:64])
                    ptr = ps_tp.tile([128, 64], FT, tag="tp", name=f"ptr{ib}")
                    nc.tensor.transpose(ptr[:], rT[:, q * 128:(q + 1) * 128],
                                        ident[0:64, 0:64])
                    fm = ep.tile([128, 64], FT, tag="fm")
                    nc.vector.tensor_scalar_max(fm[:], ptf[:], FT_CLIP)
                    omf = ep.tile([128, 64], FT, tag="omf")
                    nc.vector.tensor_scalar(omf[:], ptf[:], -1.0, 1.0,
                                            op0=mybir.AluOpType.mult,
                                            op1=mybir.AluOpType.add)
                    t1 = ep.tile([128, 64], FT, tag="t1")
                    nc.vector.tensor_mul(t1[:], fm[:],
                                         awl_sb[:, ib * 64:(ib + 1) * 64])
                    t2 = ep.tile([128, 64], FT, tag="t2")
                    nc.vector.tensor_mul(t2[:], omf[:], ptr[:])
                    wu = ep.tile([128, 64], FT, tag="wu")
                    nc.vector.tensor_add(wu[:], t1[:], t2[:])
                    nc.sync.dma_start(out_w[r0:r0 + 128, :], wu[:])
                    tpw = ps_tp.tile([64, 128], FT, tag="tp", name=f"tpw{ib}")
                    nc.tensor.transpose(tpw[:], wu[:], ident[:])
                    nc.scalar.copy(wuT[0:64, r0:r0 + 128], tpw[:])

            pP = ps_P.tile([128, 65], FT)
            for u in range(2):
                c0 = u * 512
                pz = ps_acc.tile([64, 512], FT, tag="acc", name=f"pzT{u}")
                nc.tensor.matmul(pz[:], W64["Ro"], wuT[:, c0:c0 + 512])
                zT = ep.tile([64, 512], BF, tag="zT", name=f"zT{u}")
                nc.vector.tensor_copy(zT[:], pz[:])
                for q in range(4):
                    ib = u * 4 + q
                    r0 = ib * 128
                    pzt = ps_tp.tile([128, 64], BF, tag="tp", name=f"pzt{ib}")
                    nc.tensor.transpose(pzt[:], zT[:, q * 128:(q + 1) * 128],
                                        ident_bf[0:64, 0:64])
                    z_sb = ep.tile([128, 65], BF, tag="z_sb")
                    nc.vector.tensor_copy(z_sb[:, 0:64], pzt[:])
                    nc.vector.memset(z_sb[:, 64:65], 1.0)
                    wop_t = ep.tile([128, O], BF, tag="wop")
                    nc.sync.dma_start(wop_t[:], wops[r0:r0 + 128, :])
                    a_m = ep.tile([128, O], BF, tag="am")
                    nc.vector.tensor_scalar_mul(a_m[:], wop_t[:],
                                                wesc_sb[:, ib:ib + 1])
                    nc.tensor.matmul(pP[:], a_m[:], z_sb[:],
                                     start=(ib == 0), stop=(ib == NB - 1))

            p_sb = ep.tile([128, 65], FT, tag="psb")
            nc.vector.tensor_copy(p_sb[:], pP[:])
            nc.sync.dma_start(out_p[:], p_sb[:])

    _patch_to_json_bytes(nc)
    return nc


def _get_nc():
    if "nc" not in _CACHED:
        _CACHED["nc"] = _build_nc()
    return _CACHED["nc"]


# --------------------------------------------------------------------------
# host side: shard (transpose + bf16-recode the masks), run, gather,
# tiny op epilogue
# --------------------------------------------------------------------------
def _sigmoid(x):
    return 1.0 / (1.0 + np.exp(-x))


def _wpack(inputs):
    f32 = lambda a: np.asarray(a, dtype=np.float32)
    Wf = f32(inputs["Wf"]); Wupd = f32(inputs["Wupd"])
    chunks = {
        "Rwk": (f32(inputs["Wwk"]), f32(inputs["bwk"]), 1.0),
        "Rws": (f32(inputs["Wws"]), f32(inputs["bws"]), 1.0),
        "Ro": (f32(inputs["Wo"]), f32(inputs["bo"]), 1.0),
        "G": (f32(inputs["Wg"]), f32(inputs["bg"]), 0.0),
        "F0": (Wf[0:64], f32(inputs["bf"]), 0.0),
        "F1": (Wf[64:128], None, 0.0),
        "F2": (Wf[128:192], None, 0.0),
        "F3": (Wf[192:256], None, 0.0),
        "U0": (Wupd[0:64], None, 0.0),
        "U1": (Wupd[64:128], f32(inputs["bupd"]), 0.0),
        "U2": (Wupd[128:192], None, 0.0),
    }
    img = np.zeros((65, 65 * len(_CHUNKS)), dtype=np.float32)
    for i, nm in enumerate(_CHUNKS):
        Wc, bc, one = chunks[nm]
        img[0:64, i * 65:i * 65 + 64] = Wc
        if bc is not None:
            img[64, i * 65:i * 65 + 64] = bc.reshape(-1)
        img[64, i * 65 + 64] = one
    return img


def _core_map(inputs, c, shared, bf16):
    b, s = divmod(c, 4)
    r0, r1 = s * S, (s + 1) * S
    f32 = lambda a: np.ascontiguousarray(a, dtype=np.float32)
    bfT = lambda a: np.asarray(np.asarray(a, dtype=np.float32).T,
                               dtype=bf16)
    dep = (inputs["depend_0"][b, r0:r1, :] if s < 2
           else inputs["depend_1"][b, r0 - L:r1 - L, :])
    m = {
        "adjT": bfT(inputs["word_exist_matrix"][b, r0:r1, :]),
        "wwT": bfT(inputs["word_word"][b, r0:r1, :]),
        "depT": bfT(dep),
        "wops": np.asarray(np.asarray(inputs["word_operator"][b, r0:r1, :],
                                      dtype=np.float32), dtype=bf16),
        "wesc": np.ascontiguousarray(
            np.asarray(inputs["word_exist_sequence"][b, r0:r1],
                       dtype=np.float32).reshape(NB, 128).T),
        "w0": f32(inputs["word_outputs_0"][b]),
        "w1": f32(inputs["word_outputs_1"][b]),
        "wfs": f32(inputs["word_outputs_0"][b] if s < 2
                   else inputs["word_outputs_1"][b]),
        "awl": f32(inputs["word_outputs_0"][b][r0:r1] if s < 2
                   else inputs["word_outputs_1"][b][r0 - L:r1 - L]),
        "gws": f32(np.asarray(inputs["goal_word"][b, r0:r1]).reshape(1, S)),
        "nh": f32(np.asarray(inputs["node_hidden"][b]).reshape(1, D)),
    }
    m.update(shared)
    return m


def _run(inputs, trace=False):
    import ml_dtypes
    from concourse.bass_utils import run_bass_kernel_spmd

    bf16 = ml_dtypes.bfloat16
    nc = _get_nc()
    shared = {"wpack": _wpack(inputs).astype(bf16)}

    with concurrent.futures.ThreadPoolExecutor(max_workers=8) as ex:
        in_maps = list(ex.map(lambda c: _core_map(inputs, c, shared, bf16),
                              range(N_CORES)))

    res = run_bass_kernel_spmd(nc, in_maps, core_ids=list(range(N_CORES)),
                               trace=trace)

    out = np.empty((B, N + O, D), dtype=np.float32)
    for c in range(N_CORES):
        b, s = divmod(c, 4)
        out[b, s * S:(s + 1) * S, :] = res.results[c]["out_w"]

    # host op epilogue (tiny): P summed over each batch's 4 shards
    opE = np.asarray(inputs["op_embedding"], dtype=np.float32)
    Wf2 = np.asarray(inputs["Wf2"], dtype=np.float32)
    bf2 = np.asarray(inputs["bf2"], dtype=np.float32)
    Wout = np.asarray(inputs["Wout"], dtype=np.float32)
    bout = np.asarray(inputs["bout"], dtype=np.float32)
    for b in range(B):
        P = np.zeros((128, 65), dtype=np.float32)
        for s in range(4):
            P += res.results[4 * b + s]["out_p"]
        word_op = P[:, 0:64] / (P[:, 64:65] + 1e-30)
        f2 = _sigmoid(np.concatenate([opE[b], word_op], axis=-1) @ Wf2 + bf2)
        r2 = np.maximum(word_op @ Wout + bout, 0.0)
        out[b, N:, :] = np.maximum(f2, FT_CLIP) * opE[b] + (1.0 - f2) * r2

    return out, res


def kernel(**inputs):
    out, _ = _run(inputs, trace=False)
    return out


# revision 17
# speedup vs baseline: 1.9367x; 1.1005x over previous
"""Trainium2 Bass kernel for nn_Decoder_45260365565566-family (gnn_message_passing).

Contract: kernel(**inputs) takes the FULL unsharded inputs from
reference.setup_inputs() and returns the FULL [B, N+O, D] output.

Strategy (8 NeuronCores): 2-way data parallel over batch B; within each batch
group, 4-way row-sharding of the [N, N] adjacencies (N=4096 -> 1024 rows per
core).  The sharding step hands each core its adjacency slices COLUMN-MAJOR
(transposed while slicing) and in bf16 (the masks are 0/1 -> exact recode,
halving the dominant DMA stream): the TensorEngine contracts over the
partition axis, so feeding M^T tiles directly avoids any on-device transpose
of the adjacency stream.  Each core:
  - streams its (4096 x 1024) slice of (word_word * word_exist_matrix)^T and
    its (2048 x 1024) slice of depend^T, multiplying masks on VectorE and
    accumulating  Y^T[h, i] += X^T M^T  on the TensorEngine in [65, 512]
    PSUM strips at bf16 rate (X carries a ones column so the binary
    row-degree falls out of the same matmul),
  - computes the word_g / word_w_k / word_w_s row normalisation and the
    forget/update gating with batched, weight-stationary bf16 matmuls in the
    feature-major domain, producing word_updated rows (1/8 of the output),
  - computes its partial of word_op's token contraction P = A^T @ [Z, 1].
The host sums the four [128, 65] P partials per batch (a 33 KB reduction --
an on-device AllReduce measured ~65 us on this fabric) and applies the tiny
[128, 64] op-embedding epilogue in numpy.

l2n(x/(deg+eps)) == x/(||x||+eps') up to fp32 rounding, so normalisation
happens once, after the matmul; norms and gating elementwise stay fp32.
"""

import concurrent.futures

import numpy as np

B, L, D, H, O = 2, 2048, 64, 64, 128
N = 2 * L                # 4096 tokens
S = N // 4               # 1024 rows per core
NB = S // 128            # 8 row-blocks per core
NJ = N // 128            # 32 column-blocks of the big adjacency
NJS = L // 128           # 16 column-blocks of depend
FT_CLIP = 0.2
N_CORES = 8

# packed weight image: [65, 65] column-blocks; cols 0:64 = [W; b], col 64 =
# [0...0, 1] for the "R" squares (ones column -> degree), else 0.
_CHUNKS = ["Rwk", "Rws", "Ro", "G", "F0", "F1", "F2", "F3", "U0", "U1", "U2"]

_CACHED = {}


# --------------------------------------------------------------------------
# walrus workaround: this toolchain's walrus accepts at most ONE sync-wait
# command per instruction, but Tile emits several.  Legalize the serialized
# BIR by hoisting extra waits onto NoOps inserted before the owner (engines
# run a basic block in program order, so consecutive waits AND together).
# --------------------------------------------------------------------------
def _legalize_bir(d):
    def fix_list(insts):
        out = []
        for inst in insts:
            si = inst.get("sync_info")
            if si:
                waits = si.get("on_wait") or []
                if len(waits) > 1:
                    for k, w in enumerate(waits[:-1]):
                        out.append({
                            "debug": inst.get("debug", 0),
                            "engine": inst["engine"],
                            "ins": [], "outs": [],
                            "name": f"{inst['name']}-xw{k}",
                            "opcode": "NoOp",
                            "sync_info": {"on_update": [], "on_wait": [w]},
                        })
                    si["on_wait"] = waits[-1:]
            out.append(inst)
        return out

    def walk(o):
        if isinstance(o, dict):
            if isinstance(o.get("instructions"), list):
                o["instructions"] = fix_list(o["instructions"])
            for v in o.values():
                walk(v)
        elif isinstance(o, list):
            for v in o:
                walk(v)

    walk(d)
    return d


def _patch_to_json_bytes(nc):
    import orjson
    orig = nc.to_json_bytes

    def patched():
        return orjson.dumps(_legalize_bir(orjson.loads(orig())))

    nc.to_json_bytes = patched


# --------------------------------------------------------------------------
# device program (identical on all 8 cores)
# --------------------------------------------------------------------------
def _build_nc():
    import concourse.bass as bass
    import concourse.tile as tile
    from concourse import mybir
    from concourse.masks import make_identity

    FT = mybir.dt.float32
    BF = mybir.dt.bfloat16
    ACT = mybir.ActivationFunctionType

    nc = bass.Bass(num_devices=N_CORES)

    awwT = nc.dram_tensor("awwT", [N, 2 * S], BF, kind="ExternalInput")
    depT = nc.dram_tensor("depT", [L, S], BF, kind="ExternalInput")
    wops = nc.dram_tensor("wops", [S, O], BF, kind="ExternalInput")
    wesc = nc.dram_tensor("wesc", [128, NB], FT, kind="ExternalInput")
    wpack = nc.dram_tensor("wpack", [65, 65 * len(_CHUNKS)], BF,
                           kind="ExternalInput")
    w0 = nc.dram_tensor("w0", [L, D], FT, kind="ExternalInput")
    w1 = nc.dram_tensor("w1", [L, D], FT, kind="ExternalInput")
    wfs = nc.dram_tensor("wfs", [L, D], FT, kind="ExternalInput")
    awl = nc.dram_tensor("awl", [S, D], FT, kind="ExternalInput")
    gws = nc.dram_tensor("gws", [1, S], FT, kind="ExternalInput")
    nh = nc.dram_tensor("nh", [1, D], FT, kind="ExternalInput")

    out_w = nc.dram_tensor("out_w", [S, D], FT, kind="ExternalOutput")
    out_p = nc.dram_tensor("out_p", [128, 65], FT, kind="ExternalOutput")

    with tile.TileContext(nc) as tc:
        with (
            tc.tile_pool(name="cons", bufs=1) as cons,
            tc.tile_pool(name="stream", bufs=6) as stream,
            tc.tile_pool(name="ep", bufs=2) as ep,
            tc.tile_pool(name="ps_acc", bufs=2, space="PSUM") as ps_acc,
            tc.tile_pool(name="ps_tp", bufs=3, space="PSUM") as ps_tp,
            tc.tile_pool(name="ps_ss", bufs=1, space="PSUM") as ps_ss,
            tc.tile_pool(name="ps_P", bufs=1, space="PSUM") as ps_P,
        ):
            # ===== A. stream loads + mask multiply (emitted first so the
            # DMA queues saturate from t=0; matmuls reference these later) ==
            d_tiles = []
            for j in range(NJS):
                dbf = stream.tile([128, S], BF, tag="dT", name=f"dT{j}")
                nc.sync.dma_start(dbf[:], depT[j * 128:(j + 1) * 128, :])
                d_tiles.append(dbf)
            m_tiles = []
            for j in range(NJ):
                ab = stream.tile([128, 2 * S], BF, tag="awwT", name=f"ab{j}")
                nc.sync.dma_start(ab[:], awwT[j * 128:(j + 1) * 128, :])
                mt = stream.tile([128, S], BF, tag="mT", name=f"mt{j}")
                nc.vector.tensor_mul(mt[:], ab[:, 0:S], ab[:, S:2 * S])
                m_tiles.append(mt)

            # ===== B. prep: weights, tokens, strips, X tiles ===============
            ident = cons.tile([128, 128], FT)
            make_identity(nc, ident[:])
            ident_bf = cons.tile([128, 128], BF)
            make_identity(nc, ident_bf[:])
            wp = cons.tile([65, 65 * len(_CHUNKS)], BF)
            nc.scalar.dma_start(wp[:], wpack[:])
            CH = {nm: wp[:, i * 65:i * 65 + 65] for i, nm in enumerate(_CHUNKS)}
            W64 = {nm: CH[nm][:, 0:64] for nm in _CHUNKS}

            wesc_sb = cons.tile([128, NB], FT)
            nc.scalar.dma_start(wesc_sb[:], wesc[:])

            def load_tokens(name, src, ntile):
                t = cons.tile([128, ntile * 64], FT, tag=name, name=name)
                nc.scalar.dma_start(
                    t[:].rearrange("p (t d) -> p t d", d=64),
                    src[:].rearrange("(t p) d -> p t d", p=128))
                return t

            w0_sb = load_tokens("w0_sb", w0, L // 128)
            w1_sb = load_tokens("w1_sb", w1, L // 128)
            wfs_sb = load_tokens("wfs_sb", wfs, L // 128)
            awl_sb = load_tokens("awl_sb", awl, NB)

            def bf_cast(name, src_sb, ncols):
                t = cons.tile([128, ncols * 64], BF, tag=name, name=name)
                nc.vector.tensor_copy(t[:], src_sb[:])
                return t

            w0_bf = bf_cast("w0_bf", w0_sb, L // 128)
            w1_bf = bf_cast("w1_bf", w1_sb, L // 128)
            wfs_bf = bf_cast("wfs_bf", wfs_sb, L // 128)
            awl_bf = bf_cast("awl_bf", awl_sb, NB)

            def strips65(name, src_sb, ntile):
                # [65, ntile*128] bf16 feature-major strips (+ones row)
                big = cons.tile([65, ntile * 128], BF, tag=name, name=name)
                nc.vector.memset(big[64:65, :], 1.0)
                for i in range(ntile):
                    tp = ps_tp.tile([64, 128], BF, tag="tp",
                                    name=f"tp_{name}{i}")
                    nc.tensor.transpose(tp[:], src_sb[:, i * 64:(i + 1) * 64],
                                        ident_bf[:])
                    if i % 2 == 0:
                        nc.vector.tensor_copy(
                            big[0:64, i * 128:(i + 1) * 128], tp[:])
                    else:
                        nc.scalar.copy(
                            big[0:64, i * 128:(i + 1) * 128], tp[:])
                return big

            def x_tile(name, strip, Rt):
                # [128, 65] bf16 projected+augmented features of one block
                t = cons.tile([128, 65], BF, tag=name, name=name)
                px = ps_tp.tile([128, 65], FT, tag="tp", name=f"px_{name}")
                nc.tensor.matmul(px[:], strip, Rt)
                nc.scalar.copy(t[:], px[:])
                return t

            wfsT = strips65("wfsT", wfs_bf, NJS)
            xs_tiles = [x_tile(f"xs{i}", wfsT[:, i * 128:(i + 1) * 128],
                               CH["Rws"]) for i in range(NJS)]
            awT0 = strips65("awT0", w0_bf, L // 128)
            awT1 = strips65("awT1", w1_bf, L // 128)
            xk_tiles = (
                [x_tile(f"xk{i}", awT0[:, i * 128:(i + 1) * 128], CH["Rwk"])
                 for i in range(L // 128)]
                + [x_tile(f"xkb{i}", awT1[:, i * 128:(i + 1) * 128], CH["Rwk"])
                   for i in range(L // 128)])
            awlT = strips65("awlT", awl_bf, NB)   # [65, 1024]

            ones_col = cons.tile([64, 1], FT)
            nc.vector.memset(ones_col[:], 1.0)

            # word_g^T [64, 1024] bf16 (g only enters the gate matmuls)
            nh_row = cons.tile([1, 64], FT)
            nc.scalar.dma_start(nh_row[:], nh[:])
            tpn = ps_tp.tile([64, 1], FT, tag="tp")
            nc.tensor.transpose(tpn[:], nh_row[:], ident[0:1, 0:1])
            nhT = cons.tile([65, 1], BF)
            nc.vector.tensor_copy(nhT[0:64, :], tpn[:])
            nc.vector.memset(nhT[64:65, :], 1.0)
            pgt = ps_tp.tile([1, 64], FT, tag="tp")
            nc.tensor.matmul(pgt[:], nhT[:], W64["G"])
            gt_row = cons.tile([1, 64], BF)
            nc.vector.tensor_copy(gt_row[:], pgt[:])
            gt_sq = cons.tile([1, 64], FT)
            gt_ss = cons.tile([1, 1], FT)
            nc.scalar.activation(gt_sq[:], pgt[:], ACT.Square,
                                 accum_out=gt_ss[:])
            gt_n = cons.tile([1, 1], FT)
            nc.scalar.sqrt(gt_n[:], gt_ss[:])
            gws_sb = cons.tile([1, S], FT)
            nc.scalar.dma_start(gws_sb[:], gws[:])
            den = cons.tile([1, S], FT)
            nc.vector.tensor_scalar(den[:], gws_sb[:], gt_n[:], 1e-30,
                                    op0=mybir.AluOpType.mult,
                                    op1=mybir.AluOpType.add)
            rec = cons.tile([1, S], FT)
            nc.vector.reciprocal(rec[:], den[:])
            alpha = cons.tile([1, S], BF)
            nc.vector.tensor_mul(alpha[:], rec[:], gws_sb[:])
            gT = cons.tile([64, S], BF)
            for hh in range(S // 512):
                pg = ps_tp.tile([64, 512], FT, tag="tp", name=f"pg{hh}")
                nc.tensor.matmul(pg[:], gt_row[:],
                                 alpha[:, hh * 512:(hh + 1) * 512])
                nc.vector.tensor_copy(gT[:, hh * 512:(hh + 1) * 512], pg[:])

            # ===== C/D. streamed accumulation  Y^T = X^T M^T ===============
            ps_T = [ps_acc.tile([65, 512], FT, tag="acc", name=f"psT{u}")
                    for u in range(2)]
            for j in range(NJS):
                for u in range(2):
                    nc.tensor.matmul(ps_T[u][:], xs_tiles[j][:],
                                     d_tiles[j][:, u * 512:(u + 1) * 512],
                                     start=(j == 0), stop=(j == NJS - 1))
            ysT = [ep.tile([65, 512], FT, tag=f"ysT{u}", name=f"ysT{u}")
                   for u in range(2)]
            for u in range(2):
                nc.vector.tensor_copy(ysT[u][:], ps_T[u][:])

            pk_T = [ps_acc.tile([65, 512], FT, tag="acc", name=f"pkT{u}")
                    for u in range(2)]
            for j in range(NJ):
                for u in range(2):
                    nc.tensor.matmul(pk_T[u][:], xk_tiles[j][:],
                                     m_tiles[j][:, u * 512:(u + 1) * 512],
                                     start=(j == 0), stop=(j == NJ - 1))
            ykT = [ep.tile([65, 512], FT, tag=f"ykT{u}", name=f"ykT{u}")
                   for u in range(2)]
            for u in range(2):
                nc.vector.tensor_copy(ykT[u][:], pk_T[u][:])

            # ===== E. l2 norms (pss cols: k = 0..7, s = 8..15) =============
            pss = ps_ss.tile([128, 16], FT)
            for u in range(2):
                sqk = ep.tile([64, 512], FT, tag="sqk", name=f"sqk{u}")
                nc.vector.tensor_mul(sqk[:], ykT[u][0:64, :], ykT[u][0:64, :])
                sqs = ep.tile([64, 512], FT, tag="sqs", name=f"sqs{u}")
                nc.vector.tensor_mul(sqs[:], ysT[u][0:64, :], ysT[u][0:64, :])
                for q in range(4):
                    ib = u * 4 + q
                    nc.tensor.matmul(pss[:, ib:ib + 1],
                                     sqk[:, q * 128:(q + 1) * 128],
                                     ones_col[:])
                    nc.tensor.matmul(pss[:, 8 + ib:9 + ib],
                                     sqs[:, q * 128:(q + 1) * 128],
                                     ones_col[:])
            nrm = ep.tile([128, 16], FT, tag="nrm")
            nc.scalar.sqrt(nrm[:], pss[:])
            nc.vector.tensor_scalar_add(nrm[:], nrm[:], 1e-30)
            rinv = ep.tile([128, 16], FT, tag="rinv")
            nc.vector.reciprocal(rinv[:], nrm[:])

            # token-major normalised k/s per block
            k_toks, s_toks = [], []
            for ib in range(NB):
                u, q = divmod(ib, 4)
                pyk = ps_tp.tile([128, 64], FT, tag="tp", name=f"pyk{ib}")
                nc.tensor.transpose(pyk[:],
                                    ykT[u][0:64, q * 128:(q + 1) * 128],
                                    ident[0:64, 0:64])
                kt = cons.tile([128, 64], FT, tag=f"ktok{ib}", name=f"ktok{ib}")
                nc.vector.tensor_scalar_mul(kt[:], pyk[:], rinv[:, ib:ib + 1])
                k_toks.append(kt)
                pys = ps_tp.tile([128, 64], FT, tag="tp", name=f"pys{ib}")
                nc.tensor.transpose(pys[:],
                                    ysT[u][0:64, q * 128:(q + 1) * 128],
                                    ident[0:64, 0:64])
                st = cons.tile([128, 64], FT, tag=f"stok{ib}", name=f"stok{ib}")
                nc.vector.tensor_scalar_mul(st[:], pys[:],
                                            rinv[:, 8 + ib:9 + ib])
                s_toks.append(st)

            # ===== F. feature-major strips of normalised k/s ===============
            wkT = cons.tile([65, S], BF, tag="wkT", name="wkT")
            nc.vector.memset(wkT[64:65, :], 1.0)
            wsT = cons.tile([65, S], BF, tag="wsT", name="wsT")
            nc.vector.memset(wsT[64:65, :], 1.0)
            for ib in range(NB):
                tpk = ps_tp.tile([64, 128], FT, tag="tp", name=f"tpk{ib}")
                nc.tensor.transpose(tpk[:], k_toks[ib][:], ident[:])
                nc.vector.tensor_copy(wkT[0:64, ib * 128:(ib + 1) * 128],
                                      tpk[:])
                tps = ps_tp.tile([64, 128], FT, tag="tp", name=f"tps{ib}")
                nc.tensor.transpose(tps[:], s_toks[ib][:], ident[:])
                nc.scalar.copy(wsT[0:64, ib * 128:(ib + 1) * 128], tps[:])

            # ===== G. gates, word_updated, op partials =====================
            wuT = cons.tile([65, S], BF, tag="wuT", name="wuT")
            nc.vector.memset(wuT[64:65, :], 1.0)
            for u in range(2):
                c0 = u * 512
                pf = ps_acc.tile([64, 512], FT, tag="acc", name=f"pfT{u}")
                nc.tensor.matmul(pf[:], W64["F0"], awlT[:, c0:c0 + 512],
                                 start=True, stop=False)
                nc.tensor.matmul(pf[:], W64["F1"][0:64, :],
                                 gT[:, c0:c0 + 512],
                                 start=False, stop=False)
                nc.tensor.matmul(pf[:], W64["F2"][0:64, :],
                                 wkT[0:64, c0:c0 + 512],
                                 start=False, stop=False)
                nc.tensor.matmul(pf[:], W64["F3"][0:64, :],
                                 wsT[0:64, c0:c0 + 512],
                                 start=False, stop=True)
                fT = ep.tile([64, 512], FT, tag="fT", name=f"fT{u}")
                nc.scalar.activation(fT[:], pf[:], ACT.Sigmoid)

                pn = ps_acc.tile([64, 512], FT, tag="acc", name=f"pnT{u}")
                nc.tensor.matmul(pn[:], W64["U0"][0:64, :],
                                 gT[:, c0:c0 + 512],
                                 start=True, stop=False)
                nc.tensor.matmul(pn[:], W64["U1"], wkT[:, c0:c0 + 512],
                                 start=False, stop=False)
                nc.tensor.matmul(pn[:], W64["U2"][0:64, :],
                                 wsT[0:64, c0:c0 + 512],
                                 start=False, stop=True)
                rT = ep.tile([64, 512], FT, tag="rT", name=f"rT{u}")
                nc.vector.tensor_scalar_max(rT[:], pn[:], 0.0)

                for q in range(4):
                    ib = u * 4 + q
                    r0 = ib * 128
                    ptf = ps_tp.tile([128, 64], FT, tag="tp", name=f"ptf{ib}")
                    nc.tensor.transpose(ptf[:], fT[:, q * 128:(q + 1) * 128],
                                        ident[0:64, 0:64])
                    ptr = ps_tp.tile([128, 64], FT, tag="tp", name=f"ptr{ib}")
                    nc.tensor.transpose(ptr[:], rT[:, q * 128:(q + 1) * 128],
                                        ident[0:64, 0:64])
                    fm = ep.tile([128, 64], FT, tag="fm")
                    nc.vector.tensor_scalar_max(fm[:], ptf[:], FT_CLIP)
                    omf = ep.tile([128, 64], FT, tag="omf")
                    nc.vector.tensor_scalar(omf[:], ptf[:], -1.0, 1.0,
                                            op0=mybir.AluOpType.mult,
                                            op1=mybir.AluOpType.add)
                    t1 = ep.tile([128, 64], FT, tag="t1")
                    nc.vector.tensor_mul(t1[:], fm[:],
                                         awl_sb[:, ib * 64:(ib + 1) * 64])
                    t2 = ep.tile([128, 64], FT, tag="t2")
                    nc.vector.tensor_mul(t2[:], omf[:], ptr[:])
                    wu = ep.tile([128, 64], FT, tag="wu")
                    nc.vector.tensor_add(wu[:], t1[:], t2[:])
                    nc.scalar.dma_start(out_w[r0:r0 + 128, :], wu[:])
                    tpw = ps_tp.tile([64, 128], FT, tag="tp", name=f"tpw{ib}")
                    nc.tensor.transpose(tpw[:], wu[:], ident[:])
                    nc.scalar.copy(wuT[0:64, r0:r0 + 128], tpw[:])

            pP = ps_P.tile([128, 65], FT)
            for u in range(2):
                c0 = u * 512
                pz = ps_acc.tile([64, 512], FT, tag="acc", name=f"pzT{u}")
                nc.tensor.matmul(pz[:], W64["Ro"], wuT[:, c0:c0 + 512])
                zT = ep.tile([64, 512], BF, tag="zT", name=f"zT{u}")
                nc.vector.tensor_copy(zT[:], pz[:])
                for q in range(4):
                    ib = u * 4 + q
                    r0 = ib * 128
                    pzt = ps_tp.tile([128, 64], BF, tag="tp", name=f"pzt{ib}")
                    nc.tensor.transpose(pzt[:], zT[:, q * 128:(q + 1) * 128],
                                        ident_bf[0:64, 0:64])
                    z_sb = ep.tile([128, 65], BF, tag="z_sb")
                    nc.vector.tensor_copy(z_sb[:, 0:64], pzt[:])
                    nc.vector.memset(z_sb[:, 64:65], 1.0)
                    wop_t = ep.tile([128, O], BF, tag="wop")
                    nc.scalar.dma_start(wop_t[:], wops[r0:r0 + 128, :])
                    a_m = ep.tile([128, O], BF, tag="am")
                    nc.vector.tensor_scalar_mul(a_m[:], wop_t[:],
                                                wesc_sb[:, ib:ib + 1])
                    nc.tensor.matmul(pP[:], a_m[:], z_sb[:],
                                     start=(ib == 0), stop=(ib == NB - 1))

            p_sb = ep.tile([128, 65], FT, tag="psb")
            nc.vector.tensor_copy(p_sb[:], pP[:])
            nc.scalar.dma_start(out_p[:], p_sb[:])

    _patch_to_json_bytes(nc)
    return nc


def _get_nc():
    if "nc" not in _CACHED:
        _CACHED["nc"] = _build_nc()
    return _CACHED["nc"]


# --------------------------------------------------------------------------
# host side: shard (transpose + bf16-recode the masks), run, gather,
# tiny op epilogue
# --------------------------------------------------------------------------
def _sigmoid(x):
    return 1.0 / (1.0 + np.exp(-x))


def _wpack(inputs):
    f32 = lambda a: np.asarray(a, dtype=np.float32)
    Wf = f32(inputs["Wf"]); Wupd = f32(inputs["Wupd"])
    chunks = {
        "Rwk": (f32(inputs["Wwk"]), f32(inputs["bwk"]), 1.0),
        "Rws": (f32(inputs["Wws"]), f32(inputs["bws"]), 1.0),
        "Ro": (f32(inputs["Wo"]), f32(inputs["bo"]), 1.0),
        "G": (f32(inputs["Wg"]), f32(inputs["bg"]), 0.0),
        "F0": (Wf[0:64], f32(inputs["bf"]), 0.0),
        "F1": (Wf[64:128], None, 0.0),
        "F2": (Wf[128:192], None, 0.0),
        "F3": (Wf[192:256], None, 0.0),
        "U0": (Wupd[0:64], None, 0.0),
        "U1": (Wupd[64:128], f32(inputs["bupd"]), 0.0),
        "U2": (Wupd[128:192], None, 0.0),
    }
    img = np.zeros((65, 65 * len(_CHUNKS)), dtype=np.float32)
    for i, nm in enumerate(_CHUNKS):
        Wc, bc, one = chunks[nm]
        img[0:64, i * 65:i * 65 + 64] = Wc
        if bc is not None:
            img[64, i * 65:i * 65 + 64] = bc.reshape(-1)
        img[64, i * 65 + 64] = one
    return img


def _core_map(inputs, c, shared, bf16):
    b, s = divmod(c, 4)
    r0, r1 = s * S, (s + 1) * S
    f32 = lambda a: np.ascontiguousarray(a, dtype=np.float32)
    bfT = lambda a: np.asarray(np.asarray(a, dtype=np.float32).T,
                               dtype=bf16)
    dep = (inputs["depend_0"][b, r0:r1, :] if s < 2
           else inputs["depend_1"][b, r0 - L:r1 - L, :])
    m = {
        "awwT": np.concatenate(
            [bfT(inputs["word_exist_matrix"][b, r0:r1, :]),
             bfT(inputs["word_word"][b, r0:r1, :])], axis=1),
        "depT": bfT(dep),
        "wops": np.asarray(np.asarray(inputs["word_operator"][b, r0:r1, :],
                                      dtype=np.float32), dtype=bf16),
        "wesc": np.ascontiguousarray(
            np.asarray(inputs["word_exist_sequence"][b, r0:r1],
                       dtype=np.float32).reshape(NB, 128).T),
        "w0": f32(inputs["word_outputs_0"][b]),
        "w1": f32(inputs["word_outputs_1"][b]),
        "wfs": f32(inputs["word_outputs_0"][b] if s < 2
                   else inputs["word_outputs_1"][b]),
        "awl": f32(inputs["word_outputs_0"][b][r0:r1] if s < 2
                   else inputs["word_outputs_1"][b][r0 - L:r1 - L]),
        "gws": f32(np.asarray(inputs["goal_word"][b, r0:r1]).reshape(1, S)),
        "nh": f32(np.asarray(inputs["node_hidden"][b]).reshape(1, D)),
    }
    m.update(shared)
    return m


def _run(inputs, trace=False):
    import ml_dtypes
    from concourse.bass_utils import run_bass_kernel_spmd

    bf16 = ml_dtypes.bfloat16
    nc = _get_nc()
    shared = {"wpack": _wpack(inputs).astype(bf16)}

    with concurrent.futures.ThreadPoolExecutor(max_workers=8) as ex:
        in_maps = list(ex.map(lambda c: _core_map(inputs, c, shared, bf16),
                              range(N_CORES)))

    res = run_bass_kernel_spmd(nc, in_maps, core_ids=list(range(N_CORES)),
                               trace=trace)

    out = np.empty((B, N + O, D), dtype=np.float32)
    for c in range(N_CORES):
        b, s = divmod(c, 4)
        out[b, s * S:(s + 1) * S, :] = res.results[c]["out_w"]

    # host op epilogue (tiny): P summed over each batch's 4 shards
    opE = np.asarray(inputs["op_embedding"], dtype=np.float32)
    Wf2 = np.asarray(inputs["Wf2"], dtype=np.float32)
    bf2 = np.asarray(inputs["bf2"], dtype=np.float32)
    Wout = np.asarray(inputs["Wout"], dtype=np.float32)
    bout = np.asarray(inputs["bout"], dtype=np.float32)
    for b in range(B):
        P = np.zeros((128, 65), dtype=np.float32)
        for s in range(4):
            P += res.results[4 * b + s]["out_p"]
        word_op = P[:, 0:64] / (P[:, 64:65] + 1e-30)
        f2 = _sigmoid(np.concatenate([opE[b], word_op], axis=-1) @ Wf2 + bf2)
        r2 = np.maximum(word_op @ Wout + bout, 0.0)
        out[b, N:, :] = np.maximum(f2, FT_CLIP) * opE[b] + (1.0 - f2) * r2

    return out, res


def kernel(**inputs):
    out, _ = _run(inputs, trace=False)
    return out
